# revision 58
# baseline (speedup 1.0000x reference)
"""MoE (top-2 of 8 experts, D=H=1024) on 8 Trainium2 NeuronCores.

Strategy (expert-parallel, matching the sharding hint):
  - Host computes the router (softmax + top-k + expert-sort dispatch) in
    float64 -- the dispatch/sharding decision, 0.2% of total FLOPs.
  - Tokens are gathered per expert (capacity-padded); core c gets expert c's
    token block plus expert c's weights.
  - Each core runs the 2-layer expert MLP in "transposed activation" layout
    (activations are [feature, token]) so no on-device transposes are needed:
        hT = w_in.T @ xT   (lhsT = w_in chunk, natural layout)
        yT = w_out.T @ hT  (lhsT = w_out chunk, natural layout)
  - Schedule built for overlap:
      * w_in streamed in 8 single-k-chunk DMAs on the sync queue, interleaved
        with xT chunks on the scalar queue, in the order layer-1 k-rounds
        consume them.
      * Layer 1 is k-outer (8 concurrent PSUM groups); the last two k-rounds
        run per-m pairs so each PSUM group closes early and its gelu can
        start while remaining pairs compute.
      * Layer 2 runs in 4 m-phases (k-outer inside a phase). Each phase's
        PSUM banks are drained (DVE copy, fp32->fp16) and DMA'd to HBM while
        the next phase computes, so only the last phase's drain is exposed.
      * w_out is loaded in column halves (m 0-3 then m 4-7) so phase 0 only
        waits on half the layer-2 weight bytes.
  - Host scales rows by the gate probability (zero for padding rows), adds
    b_out, and scatter-adds back into the [T, D] output.
"""

import os
import sys

import numpy as np

for _p in ("/opt/trn_rl_repo", "/root/.axon_site/_ro/trn_rl_repo"):
    if os.path.isdir(_p) and _p not in sys.path:
        sys.path.append(_p)


def _ensure_ntff_hook():
    """Register the axon NTFF profiling hook if the image's antenv lacks it."""
    try:
        import antenv.axon_hooks  # noqa: F401

        return
    except ImportError:
        pass
    import types

    try:
        import antenv
    except ImportError:
        return
    mod = types.ModuleType("antenv.axon_hooks")
    _hook = [None]
    mod.set_axon_ntff_profile_hook = lambda h: _hook.__setitem__(0, h)
    mod.get_axon_ntff_profile_hook = lambda: _hook[0]
    sys.modules["antenv.axon_hooks"] = mod
    antenv.axon_hooks = mod
    try:
        from trn_agent_boot.trn_boot import _ntff_profile_via_ctypes

        mod.set_axon_ntff_profile_hook(
            _ntff_profile_via_ctypes("/opt/axon/libaxon_pjrt.so")
        )
    except Exception:
        pass


_ensure_ntff_hook()

D, H, E, TOPK = 1024, 1024, 8, 2
N_CORES = 8
P = 128  # partitions

# Matmul input dtype: float32 (exact, 4 cyc/row), float32r (1 cyc/row,
# ~13-bit multiplies), float16 / bfloat16 (1 cyc/row + fast weight load,
# half the weight DMA bytes).
MM_DTYPE = os.environ.get("MOE_MM_DTYPE", "float16")
NWARM = int(os.environ.get("MOE_NWARM", "66"))
CANARY = os.environ.get("MOE_CANARY", "0") == "1"

_compiled_cache = {}


def _np_mm_dtype(mm_dtype_str):
    if mm_dtype_str in ("float32", "float32r"):
        return np.float32
    if mm_dtype_str == "float16":
        return np.float16
    if mm_dtype_str == "bfloat16":
        import ml_dtypes

        return np.dtype(ml_dtypes.bfloat16)
    raise ValueError(mm_dtype_str)


def _build_program(C, mm_dtype_str):
    """One expert's MLP over a [C] token block; same program on all cores."""
    from concourse import bacc, mybir, tile

    f32 = mybir.dt.float32
    f16 = mybir.dt.float16
    mm_dt = getattr(mybir.dt, mm_dtype_str)
    nc = bacc.Bacc(None, target_bir_lowering=False, debug=False)

    KD = D // P  # contraction chunks, layer 1 (and output chunks, layer 2)
    KH = H // P

    # All DRAM tensors are host-packed partition-major so every DMA moves
    # maximal contiguous runs per partition (4KB+ descriptors).  Small
    # strided descriptors (<= 2KB) roughly halve effective DMA bandwidth.
    xT_d = nc.dram_tensor("xT", [P, KD * C], mm_dt, kind="ExternalInput")
    w_in_d = nc.dram_tensor("w_in", [P, KD * H], mm_dt, kind="ExternalInput")
    # w_out packed as [P, col-half, k, 512]
    w_out_d = nc.dram_tensor("w_out", [P, 2 * KH * 512], mm_dt, kind="ExternalInput")
    bias_d = nc.dram_tensor("bias", [P, KH], f32, kind="ExternalInput")
    yT_d = nc.dram_tensor("yT", [P, KD * C], f16, kind="ExternalOutput")

    with tile.TileContext(nc) as tc:
        with (
            tc.tile_pool(name="wpool", bufs=1) as wpool,
            tc.tile_pool(name="xpool", bufs=1) as xpool,
            tc.tile_pool(name="hpool", bufs=1) as hpool,
            tc.tile_pool(name="ypool", bufs=1) as ypool,
            tc.tile_pool(name="bpool", bufs=1) as bpool,
            tc.tile_pool(name="psum", bufs=4, space="PSUM") as pspool,
        ):
            w1 = wpool.tile([P, KD, H], mm_dt, tag="w1")
            xt = xpool.tile([P, KD, C], mm_dt, tag="xt")
            w2 = wpool.tile([P, 2, KH, 512], mm_dt, tag="w2")
            bias = bpool.tile([P, KH], f32, tag="bias")

            xT_r = xT_d.rearrange("p (k c) -> p k c", k=KD)
            w1_r = w_in_d.rearrange("p (k h) -> p k h", k=KD)
            w2_r = w_out_d.rearrange("p (s k d) -> p s k d", s=2, k=KH)

            # DMA schedule. Issues cost ~650-780ns of sequencer time each and
            # transfers drain FIFO through the shared DMA engines (~360 GB/s
            # per core), so issue-completion order IS transfer order.  Put w1
            # k-chunks as singles on sync (first-needed first), xT chunks on
            # scalar so they interleave between w1 chunks, and w_out last on
            # sync as quarter-column loads (all k, 256 output cols each) so
            # layer-2 phase j only waits for its own quarter.
            # gpsimd canaries: tiny ops gated on each DMA's completion sem;
            # their trace timestamps reveal actual chunk arrival times
            if CANARY:
                cpool = bpool.tile([P, 64], mm_dt, tag="canary")
                _cn = [0]

                def canary(src_region):
                    nc.gpsimd.tensor_copy(cpool[:, _cn[0] : _cn[0] + 1], src_region)
                    _cn[0] += 1
            else:
                def canary(src_region):
                    pass

            # w1 on sync: singles up front (smooth per-round pacing), doubles
            # at the tail (the DGE allows only ~6-7 outstanding issues; more
            # issues queued before the last w1 chunk delays its transfer).
            # w_out follows on the same queue in (col-half, k-half) blocks so
            # its 2MB cannot cut ahead of w1 in the shared transfer pipe.
            for k0, k1 in ((0, 1), (1, 2), (2, 3), (3, 4), (4, 6), (6, 8)):
                nc.sync.dma_start(w1[:, k0:k1, :], w1_r[:, k0:k1, :])
                canary(w1[:, k1 - 1, 0:1])
            for s, kk0, kk1 in ((0, 0, 4), (0, 4, 8), (1, 0, 4), (1, 4, 8)):
                nc.sync.dma_start(w2[:, s, kk0:kk1, :], w2_r[:, s, kk0:kk1, :])
                canary(w2[:, s, kk1 - 1, 511:512])

            # xT pieces slot between w1 chunks in the transfer pipe, in the
            # order layer-1 rounds consume them
            for kk0, kk1 in ((0, 2), (2, 4), (4, KD)):
                nc.scalar.dma_start(xt[:, kk0:kk1, :], xT_r[:, kk0:kk1, :])
                canary(xt[:, kk1 - 1, 0:1])
            # bias is tiny and only needed by the first gelu (~18us)
            nc.scalar.dma_start(bias[:], bias_d[:])

            gelu = mybir.ActivationFunctionType.Gelu_apprx_tanh

            # PE warmup while the first weight/activation chunks stream in:
            # keeps the PE continuously busy from engine-release (~7.2us)
            # until the first real matmul's inputs land so the HAM clock gate
            # is ramping the whole time.  Skinny shapes (32-col stationary,
            # 64-col moving) keep warmup SBUF traffic low so the concurrent
            # weight DMA runs near idle rate.
            wz = bpool.tile([P, P], mm_dt, tag="wz")
            nc.vector.memset(wz[:], 0.0)
            psw = pspool.tile([P, 2, 512], f32, tag="ps", name="ps_warm")
            # skinny warmup shapes (32-col stationary, 64-col moving): low
            # SBUF traffic so the concurrent w1/xT DMA runs near idle rate
            for i in range(NWARM):
                nc.tensor.matmul(
                    psw[:32, 0, :64], wz[:, :32], wz[:, :64], start=True, stop=True
                )

            # layer 1: k-outer rounds 0..5 feed 8 concurrent PSUM groups as
            # weight chunks land; rounds 6+7 run per-m pairs so each group
            # closes early and its gelu starts while later pairs compute.
            ht = hpool.tile([P, KH, C], mm_dt, tag="ht")
            ps1 = [
                pspool.tile([P, 2, 512], f32, tag="ps", name=f"ps1_{i}")
                for i in range(KH // 2)
            ]
            for k in range(KD - 2):
                for m in range(KH):
                    nc.tensor.matmul(
                        ps1[m // 2][:, m % 2, :C],
                        w1[:, k, m * P : (m + 1) * P],
                        xt[:, k, :],
                        start=(k == 0),
                        stop=False,
                    )
            for m in range(KH):
                for k in (KD - 2, KD - 1):
                    nc.tensor.matmul(
                        ps1[m // 2][:, m % 2, :C],
                        w1[:, k, m * P : (m + 1) * P],
                        xt[:, k, :],
                        start=False,
                        stop=(k == KD - 1),
                    )
                nc.scalar.activation(
                    ht[:, m, :],
                    ps1[m // 2][:, m % 2, :C],
                    gelu,
                    bias=bias[:, m : m + 1],
                )

            # layer 2: m-phases (2,2,2,1,1 output chunks), k-outer inside
            # each; drain each phase's PSUM to SBUF (fp32->fp16 cast on DVE)
            # and DMA out from gpsimd (idle queue) while the next phase
            # computes.  The last two single-chunk phases keep the exposed
            # tail drain small.
            yt = ypool.tile([P, KD, C], f16, tag="yt")
            # five PSUM tiles: m6 and m7 get separate tiles (ps2_4 aliases
            # ps2_0's banks, long since drained) so the m6 drain never gates
            # m7's matmuls via a same-tile dependency - only the final
            # single-chunk drain is exposed after the last matmul
            ps2 = [
                pspool.tile([P, 2, 512], f32, tag="ps", name=f"ps2_{i}")
                for i in range(5)
            ]
            tile_of = {0: (0, 0), 1: (0, 1), 2: (1, 0), 3: (1, 1),
                       4: (2, 0), 5: (2, 1), 6: (3, 0), 7: (4, 0)}
            yT_r = yT_d.rearrange("p (m c) -> p m c", m=KD)
            phases = [(0, 1), (2, 3), (4, 5), (6,), (7,)]
            for ms in phases:
                for k in range(KH):
                    for m in ms:
                        tj, th = tile_of[m]
                        nc.tensor.matmul(
                            ps2[tj][:, th, :C],
                            w2[:, m // 4, k, (m % 4) * P : (m % 4 + 1) * P],
                            ht[:, k, :],
                            start=(k == 0),
                            stop=(k == KH - 1),
                        )
                m0, m1 = ms[0], ms[-1] + 1
                tj, th = tile_of[m0]
                nc.vector.tensor_copy(
                    yt[:, m0:m1, :], ps2[tj][:, th : th + (m1 - m0), :C]
                )
                nc.sync.dma_start(yT_r[:, m0:m1, :], yt[:, m0:m1, :])

    nc.compile()
    if not nc.is_finalized():
        nc.finalize()
    return nc


def _get_program(C):
    key = (C, MM_DTYPE)
    if key not in _compiled_cache:
        _compiled_cache[key] = _build_program(C, MM_DTYPE)
    return _compiled_cache[key]


def _route(x2, router_w):
    """Host router in float64: top-2 experts + gate probs per token."""
    logits = x2.astype(np.float64) @ np.asarray(router_w, np.float64)
    logits -= logits.max(axis=-1, keepdims=True)
    ex = np.exp(logits)
    probs = ex / ex.sum(axis=-1, keepdims=True)
    top_e = np.argsort(-probs, axis=-1, kind="stable")[:, :TOPK]  # [T, K]
    top_p = np.take_along_axis(probs, top_e, axis=-1)  # [T, K]
    return top_e, top_p.astype(np.float32)


def kernel(input_batch, router_w, w_in, b_in, w_out, b_out, run_kwargs=None):
    from concourse.bass_utils import run_bass_kernel_spmd

    x = np.ascontiguousarray(np.asarray(input_batch, np.float32))
    B, S, Dm = x.shape
    T = B * S
    x2 = x.reshape(T, Dm)

    top_e, top_p = _route(x2, router_w)

    # per-expert dispatch lists, in expert-sorted (token, k) order like the
    # reference's stable argsort over flattened (token, k) pairs
    tok_lists = [[] for _ in range(E)]
    p_lists = [[] for _ in range(E)]
    for t in range(T):
        for j in range(TOPK):
            e = top_e[t, j]
            tok_lists[e].append(t)
            p_lists[e].append(top_p[t, j])

    counts = [len(l) for l in tok_lists]
    # capacity per wave; a PSUM bank caps the matmul free dim at 512, so an
    # expert with >512 routed tokens (never happens for the spec'd input
    # distribution) is processed in multiple SPMD waves
    n_waves = max(1, -(-max(counts) // 512))
    if n_waves == 1:
        C = max(256, -(-max(counts) // 8) * 8)  # multiple of 8
    else:
        C = 512

    nc = _get_program(C)
    mm_np = _np_mm_dtype(MM_DTYPE)

    w_in = np.asarray(w_in, np.float32)
    w_out = np.asarray(w_out, np.float32)
    b_in = np.asarray(b_in, np.float32)
    b_out = np.asarray(b_out, np.float32)

    KD, KH = D // P, H // P
    # partition-major packed weights (see _build_program layout comments)
    w_in_packed = [
        np.ascontiguousarray(
            w_in[e].astype(mm_np).reshape(KD, P, H).transpose(1, 0, 2).reshape(P, KD * H)
        )
        for e in range(E)
    ]
    w_out_packed = [
        np.ascontiguousarray(
            w_out[e]
            .astype(mm_np)
            .reshape(KH, P, 2, 512)
            .transpose(1, 2, 0, 3)
            .reshape(P, 2 * KH * 512)
        )
        for e in range(E)
    ]
    bias_packed = [
        np.ascontiguousarray(b_in[e].reshape(KH, P).T) for e in range(E)
    ]

    out = np.zeros((T, Dm), np.float32)
    for w in range(n_waves):
        in_maps = []
        for e in range(E):
            idx = np.asarray(tok_lists[e][w * C : (w + 1) * C], np.int64)
            xT = np.zeros((KD, P, C), mm_np)
            if len(idx):
                xT.reshape(D, C)[:, : len(idx)] = x2[idx].T.astype(mm_np)
            in_maps.append(
                {
                    "xT": np.ascontiguousarray(
                        xT.transpose(1, 0, 2).reshape(P, KD * C)
                    ),
                    "w_in": w_in_packed[e],
                    "w_out": w_out_packed[e],
                    "bias": bias_packed[e],
                }
            )

        res = run_bass_kernel_spmd(
            nc, in_maps, core_ids=list(range(N_CORES)), **(run_kwargs or {})
        )
        kernel.last_results = res

        for e in range(E):
            idx = np.asarray(tok_lists[e][w * C : (w + 1) * C], np.int64)
            n = len(idx)
            if n == 0:
                continue
            p = np.asarray(p_lists[e][w * C : (w + 1) * C], np.float32)
            # unpack [P, KD*C] -> [D, C]
            yT = (
                res.results[e]["yT"].reshape(P, KD, C).transpose(1, 0, 2).reshape(D, C)
            )
            y = (yT[:, :n].T.astype(np.float32) + b_out[e]) * p[:, None]
            np.add.at(out, idx, y)

    return out.reshape(B, S, Dm)
